# revision 1
# baseline (speedup 1.0000x reference)
"""Trainium2 Bass kernel for 2D block-local multi-head attention.

Problem (hardcoded): x [1,128,48,64] -> 3x3 conv projections to q/k/v
(d_model=32, 8 heads, d_head=4), t2t local_attention_2d with
query_shape=(128,24), memory_flange=(8,8), combine heads, 3x3 output conv.

Key structural facts exploited:
  * H=128, W=48, query blocks 128x24 -> exactly 2 blocks (nH=1, nW=2).
  * The memory flange (8 px each side) is entirely zero padding, which the
    reference masks with -1e9 (exp -> exactly 0 in fp32). So each block's
    effective key set is the static 128x32 strip of ORIGINAL pixels:
    block b queries = cols [24b, 24b+24), keys = cols [16b, 16b+32).
  * Softmax without max-subtraction is safe: logits are O(10), and bf16
    shares the fp32 exponent range, so exp cannot overflow.

Sharding: one head per NeuronCore (8 heads / 8 cores), zero cross-core
communication. Each core computes q/k/v for its head (full image), block-local
attention, and a partial output conv (contraction over its 4 head channels).
Host sums the 8 partial [64, 6144] results.

Conv trick: with channel-major tensors stored on the padded 130x50 grid, a
3x3 tap (dh, dw) is a pure flat-index shift of dh*50+dw, so the matmul RHS is
one contiguous run over padded output positions (matmul RHS must be 1-D);
the junk columns (c=48, 49) are dropped in the psum->SBUF copy. The output
conv additionally packs tap pairs (dh,0)+(dh,1) into one 8-partition
contraction using a copy of o^T pre-shifted by one column on partitions 4:8.

Attention layouts (channel-on-partition):
  logitsT psum [128 keys, G queries] = K_kt.T @ Q  (keys on partitions)
  exp tile (bf16) -> AV matmul:  av[8, q] += V'_kt.T @ exp_kt, where V'_kt
  [128 keys, 8] holds v in cols 0:4 and 1.0 in cols 4:8 (denominator rows).
Matmuls use float32r (full-rate fp32) for convs and bf16 for attention.
"""

import numpy as np

H, W, CIN, DM, NH, DH = 128, 48, 64, 32, 8, 4
HP, WP = H + 2, W + 2          # padded spatial dims for 3x3 SAME conv
PADN = HP * WP + 4             # padded flat buffer size (+4 tail overrun slack)
NPIX = H * W                   # 6144
QW, KW = 24, 32                # per-block query/key column widths
NQ = H * QW                    # 3072 queries per block
NK = H * KW                    # 4096 keys per block
NKT = 32                       # key tiles (128 keys each) per block
G = 1024                       # query granule (psum/ACT tile width)
NG = NQ // G                   # 3
CHUNK_ROWS = 8                 # conv output rows per matmul chunk
NCHUNK = H // CHUNK_ROWS       # 16
CN = CHUNK_ROWS * WP           # conv matmul free size (padded-width run), 400

_cached = {}


def _build_nc():
    import concourse.bacc as bacc
    import concourse.tile as tile
    import concourse.mybir as mybir

    f32 = mybir.dt.float32
    f32r = mybir.dt.float32r    # fp32 data, full-rate PE (reduced mul precision)
    bf16 = mybir.dt.bfloat16

    nc = bacc.Bacc("TRN2", target_bir_lowering=False)

    xx_d = nc.dram_tensor("xx", [128, PADN], bf16, kind="ExternalInput")
    wqkv_d = nc.dram_tensor("wqkv", [128, 6 * 12], bf16, kind="ExternalInput")
    bias_d = nc.dram_tensor("bias12", [12, 1], f32, kind="ExternalInput")
    wo2_d = nc.dram_tensor("wo2", [8, 3 * 64], f32r, kind="ExternalInput")
    wo1_d = nc.dram_tensor("wo1", [DH, 3 * 64], f32r, kind="ExternalInput")
    id4_d = nc.dram_tensor("id4", [DH, DH], bf16, kind="ExternalInput")
    zp_d = nc.dram_tensor("zp", [DH, PADN], f32r, kind="ExternalInput")
    outp_d = nc.dram_tensor("outp", [CIN, NPIX], f32, kind="ExternalOutput")

    with tile.TileContext(nc) as tc:
        with tc.tile_pool(name="main", bufs=1) as mp:
            xx = mp.tile([128, PADN], bf16)
            wqkv = mp.tile([128, 6 * 12], bf16)
            bias12 = mp.tile([12, 1], f32)
            wo2 = mp.tile([8, 3 * 64], f32r)
            wo1 = mp.tile([DH, 3 * 64], f32r)
            id4 = mp.tile([DH, DH], bf16)
            xx_ap = xx_d.ap()
            for q4 in range(4):
                s4 = (PADN // 4) * q4
                e4 = PADN if q4 == 3 else (PADN // 4) * (q4 + 1)
                nc.sync.dma_start(xx[:, s4:e4], xx_ap[:, s4:e4])
            nc.sync.dma_start(wqkv[:], wqkv_d.ap())
            nc.sync.dma_start(bias12[:], bias_d.ap())
            nc.sync.dma_start(wo2[:], wo2_d.ap())
            nc.sync.dma_start(wo1[:], wo1_d.ap())
            nc.sync.dma_start(id4[:], id4_d.ap())

            qkvT = mp.tile([12, NPIX], bf16)
            kTf = mp.tile([DH, NPIX], bf16)      # k^T spatial (DMA dest)
            vT = mp.tile([DH, NPIX], bf16)       # v^T spatial (DMA dest)
            kTb = mp.tile([DH, 2 * NK], bf16)    # block-contiguous key strips
            vTb = mp.tile([DH, 2 * NK], bf16)    # block-contiguous value strips
            qb = mp.tile([DH, 2 * NQ], bf16)     # block-contiguous queries
            vp = mp.tile([128, 2 * NKT * 8], bf16)  # V' tiles, ones in cols 4:8
            av_sb = mp.tile([8, 2 * NQ], f32)    # rows 0:4 unnorm o, 4:8 den
            ebias = mp.tile([128, 1], f32)       # exp input shift (overflow guard)
            actwarm = mp.tile([128, 1], f32)     # dummy exp target (table preload)
            pewarm = mp.tile([DH, 512], bf16)    # garbage src for PE HAM warmup
            den4 = mp.tile([DH, NQ], f32)        # per-block denominator staging
            oo = mp.tile([8, PADN], f32r)        # padded o^T; rows 4:8 = col+1

            # ---- q/k/v projections: 3x3 conv, tap pairs (dh,0)+(dh,1)
            # packed on 128 partitions (xx rows 64:128 are shifted by +1 col),
            # plus (dh,2) singles; bias added in the psum->SBUF copy.
            # Repacks/V' transposes are emitted per chunk, interleaved with the
            # conv, so the in-order DVE stream overlaps them with later chunks.
            nc.vector.memset(vp[:], 1.0)
            nc.vector.memset(ebias[:], -30.0)
            # dummy exp at t~0: pulls the ~2.7us ACT exp-table load off the
            # critical path (it would otherwise fire at the first real exp,
            # right when the attention pipeline starts)
            nc.scalar.activation(
                actwarm[:], ebias[:], mybir.ActivationFunctionType.Exp,
                bias=ebias[:],
            )
            # ~3.4us of dummy matmuls during the DMA-in window: drives the PE
            # HAM activity monitor to full clock (2.4GHz) before the conv, and
            # absorbs the cold-clock warmup in otherwise-idle PE time. Reads
            # uninitialized SBUF (never written - no deps), result unused.
            with tc.tile_pool(name="wps", bufs=1, space="PSUM") as wps:
                nc.vector.memset(pewarm[:], 1.0)
                wp = wps.tile([DH, 512], f32, tag="wp")
                for _ in range(6):
                    nc.tensor.matmul(wp[:], pewarm[:, 0:DH], pewarm[:],
                                     start=True, stop=True)
            qT_v = qkvT[0:4, :].rearrange("p (h w) -> p h w", w=W)
            qb_v = qb[:].rearrange("p (b h w) -> p b h w", b=2, w=QW)
            kT_v = kTf[:].rearrange("p (h w) -> p h w", w=W)
            kb_v = kTb[:].rearrange("p (b h w) -> p b h w", b=2, w=KW)
            vT_v = vT[:].rearrange("p (h w) -> p h w", w=W)
            vb_v = vTb[:].rearrange("p (b h w) -> p b h w", b=2, w=KW)
            with (
                tc.tile_pool(name="cps", bufs=2, space="PSUM") as cps,
                tc.tile_pool(name="tps", bufs=4, space="PSUM") as tps,
            ):
                for ci in range(NCHUNK):
                    ps = cps.tile([12, CN], f32, tag="cps")
                    f0 = ci * CHUNK_ROWS * WP
                    for dh in range(3):
                        s = f0 + dh * WP
                        nc.tensor.matmul(
                            ps[:], wqkv[:, 12 * dh:12 * (dh + 1)],
                            xx[:, s:s + CN],
                            start=(dh == 0), stop=False,
                        )
                        nc.tensor.matmul(
                            ps[:], wqkv[0:CIN, 36 + 12 * dh:36 + 12 * (dh + 1)],
                            xx[0:CIN, s + 2:s + 2 + CN],
                            start=False, stop=(dh == 2),
                        )
                    # bias add + drop the 2 junk columns (cast to bf16)
                    psv = ps[:].rearrange("p (r c) -> p r c", c=WP)
                    nc.vector.tensor_scalar_add(
                        qkvT[:, ci * CHUNK_ROWS * W:(ci + 1) * CHUNK_ROWS * W],
                        psv[:, :, 0:W], bias12[:])
                    r0 = ci * CHUNK_ROWS
                    rs = slice(r0 * W, (r0 + CHUNK_ROWS) * W)
                    nc.sync.dma_start(kTf[:, rs], qkvT[4:8, rs])
                    nc.sync.dma_start(vT[:, rs], qkvT[8:12, rs])
                    rr = slice(r0, r0 + CHUNK_ROWS)
                    for b in range(2):
                        nc.vector.tensor_copy(
                            qb_v[:, b, rr], qT_v[:, rr, QW * b:QW * b + QW])
                        nc.vector.tensor_copy(
                            kb_v[:, b, rr], kT_v[:, rr, 16 * b:16 * b + KW])
                        nc.vector.tensor_copy(
                            vb_v[:, b, rr], vT_v[:, rr, 16 * b:16 * b + KW])
                        for kt in (2 * ci, 2 * ci + 1):
                            ps2 = tps.tile([128, DH], bf16, tag="tps")
                            nc.tensor.transpose(
                                ps2[:],
                                vTb[:, b * NK + 128 * kt:b * NK + 128 * (kt + 1)],
                                id4[:],
                            )
                            base = (b * NKT + kt) * 8
                            nc.scalar.copy(vp[:, base:base + 4], ps2[:])

            # padded o^T borders zeroed while attention runs
            nc.sync.dma_start(oo[0:4, :], zp_d.ap())

            # ---- attention + per-block normalization ----
            oo_v = oo[0:DH, 0:HP * WP].rearrange("p (h w) -> p h w", w=WP)
            with (
                tc.tile_pool(name="lgp", bufs=3, space="PSUM") as lgp,
                tc.tile_pool(name="avp", bufs=1, space="PSUM") as avp,
                tc.tile_pool(name="exp", bufs=3) as exp_pool,
            ):
                def norm_half(b, hf):
                    # normalize rows [64*hf, 64*hf+64) of block b:
                    # o = unnorm / den, written into the padded o^T grid
                    HQ = NQ // 2
                    sl = slice(b * NQ + hf * HQ, b * NQ + (hf + 1) * HQ)
                    dn = den4[:, hf * HQ:(hf + 1) * HQ]
                    nc.sync.dma_start(dn, av_sb[4:8, sl])
                    nc.vector.reciprocal(dn, dn)
                    nc.vector.tensor_mul(av_sb[0:4, sl], av_sb[0:4, sl], dn)
                    on_v = av_sb[0:4, sl].rearrange("p (h w) -> p h w", w=QW)
                    r0 = hf * (H // 2)
                    nc.vector.tensor_copy(
                        oo_v[:, 1 + r0:1 + r0 + H // 2,
                             1 + QW * b:1 + QW * b + QW], on_v)

                for b in range(2):
                    for g in range(NG):
                        q0 = b * NQ + g * G
                        av = avp.tile([8, G], f32, tag="av")
                        for kt in range(NKT):
                            lg = lgp.tile([128, G], f32, tag="lg")
                            kap = kTb[:, b * NK + 128 * kt:b * NK + 128 * (kt + 1)]
                            for j in range(G // 512):
                                nc.tensor.matmul(
                                    lg[:, 512 * j:512 * (j + 1)],
                                    kap,
                                    qb[:, q0 + 512 * j:q0 + 512 * (j + 1)],
                                    start=True, stop=True,
                                )
                            ex = exp_pool.tile([128, G], bf16, tag="ex")
                            # bias shifts exp's overflow window to logits in
                            # (-57, +118) at zero cost (free affine stage);
                            # num/den scale identically so the result is exact
                            nc.scalar.activation(
                                ex[:], lg[:],
                                mybir.ActivationFunctionType.Exp,
                                bias=ebias[:],
                            )
                            vbase = (b * NKT + kt) * 8
                            for j in range(G // 512):
                                nc.tensor.matmul(
                                    av[:, 512 * j:512 * (j + 1)],
                                    vp[:, vbase:vbase + 8],
                                    ex[:, 512 * j:512 * (j + 1)],
                                    start=(kt == 0), stop=(kt == NKT - 1),
                                )
                        nc.vector.tensor_copy(av_sb[:, q0:q0 + G], av[:])
                        # rows [0,64) are covered by granules 0-1; rows
                        # [64,128) by granules 1-2 -> normalize early
                        if g == 1:
                            norm_half(b, 0)
                        elif g == 2:
                            norm_half(b, 1)

            # shifted copy for tap pairing: oo[4:8, c] = oo[0:4, c+1],
            # chunked by row-halves so the output conv can start early
            HF = (1 + H // 2) * WP
            nc.sync.dma_start(oo[4:8, 0:HF], oo[0:4, 1:HF + 1])
            nc.sync.dma_start(oo[4:8, HF:PADN - 1], oo[0:4, HF + 1:PADN])

            # ---- output conv (partial over this head's 4 channels) ----
            # tap pairs (dh,0)+(dh,1) via 8-partition contraction + (dh,2) singles
            outp_ap = outp_d.ap()
            with (
                tc.tile_pool(name="ops", bufs=2, space="PSUM") as ops,
                tc.tile_pool(name="ost", bufs=3) as ost,
            ):
                for ci in range(NCHUNK):
                    ps = ops.tile([CIN, CN], f32, tag="ops")
                    f0 = ci * CHUNK_ROWS * WP
                    for dh in range(3):
                        s = f0 + dh * WP
                        nc.tensor.matmul(
                            ps[:], wo2[:, 64 * dh:64 * (dh + 1)], oo[:, s:s + CN],
                            start=(dh == 0), stop=False,
                        )
                        nc.tensor.matmul(
                            ps[:], wo1[:, 64 * dh:64 * (dh + 1)],
                            oo[0:4, s + 2:s + 2 + CN],
                            start=False, stop=(dh == 2),
                        )
                    psv = ps[:].rearrange("p (r c) -> p r c", c=WP)
                    stage = ost.tile([CIN, CHUNK_ROWS * W], f32, tag="ost")
                    nc.vector.tensor_copy(stage[:], psv[:, :, 0:W])
                    nc.sync.dma_start(
                        outp_ap[:, ci * CHUNK_ROWS * W:(ci + 1) * CHUNK_ROWS * W],
                        stage[:])

    nc.compile()
    return nc


def _prep_inputs(x, wq, bq, wk, bk, wv, bv, wo):
    f32 = np.float32
    x = np.ascontiguousarray(np.asarray(x, f32))
    scale = f32(DH) ** -0.5

    bf = ml_bf16()
    xx = np.zeros((128, PADN), np.float32)
    xv = xx[:CIN, :HP * WP].reshape(CIN, HP, WP)
    xv[:, 1:1 + H, 1:1 + W] = x[0].transpose(2, 0, 1)
    xx[CIN:, :PADN - 1] = xx[:CIN, 1:]
    xx = xx.astype(bf)

    wq = np.asarray(wq, f32) * scale
    bq = np.asarray(bq, f32) * scale
    wk = np.asarray(wk, f32)
    bk = np.asarray(bk, f32)
    wv = np.asarray(wv, f32)
    bv = np.asarray(bv, f32)
    wo = np.asarray(wo, f32)

    id4 = np.eye(DH, dtype=ml_bf16())
    zp = np.zeros((DH, PADN), f32)
    in_maps = []
    for h in range(NH):
        sl = slice(4 * h, 4 * h + 4)
        wqkv = np.zeros((128, 6, 12), f32)
        for dh in range(3):
            for p, dw in ((0, 0), (1, 1)):   # pair slots on partition halves
                wqkv[64 * p:64 * p + CIN, dh, 0:4] = wq[dh, dw, :, sl]
                wqkv[64 * p:64 * p + CIN, dh, 4:8] = wk[dh, dw, :, sl]
                wqkv[64 * p:64 * p + CIN, dh, 8:12] = wv[dh, dw, :, sl]
            wqkv[:CIN, 3 + dh, 0:4] = wq[dh, 2, :, sl]
            wqkv[:CIN, 3 + dh, 4:8] = wk[dh, 2, :, sl]
            wqkv[:CIN, 3 + dh, 8:12] = wv[dh, 2, :, sl]
        bias12 = np.concatenate([bq[sl], bk[sl], bv[sl]]).reshape(12, 1)
        wo2 = np.zeros((8, 3, 64), f32)
        wo1 = np.zeros((DH, 3, 64), f32)
        for dh in range(3):
            wo2[0:4, dh] = wo[dh, 0, sl, :]
            wo2[4:8, dh] = wo[dh, 1, sl, :]
            wo1[:, dh] = wo[dh, 2, sl, :]
        in_maps.append({
            "xx": xx,
            "bias12": np.ascontiguousarray(bias12.astype(f32)),
            "wqkv": np.ascontiguousarray(wqkv.reshape(128, 6 * 12).astype(bf)),
            "wo2": np.ascontiguousarray(wo2.reshape(8, 3 * 64)),
            "wo1": np.ascontiguousarray(wo1.reshape(DH, 3 * 64)),
            "id4": id4,
            "zp": zp,
        })
    return in_maps


def ml_bf16():
    import ml_dtypes
    return ml_dtypes.bfloat16


def _run(in_maps, trace=False, trace_cores=None):
    from concourse.bass_utils import run_bass_kernel_spmd

    if "nc" not in _cached:
        _cached["nc"] = _build_nc()
    return run_bass_kernel_spmd(
        _cached["nc"], in_maps, core_ids=list(range(NH)),
        trace=trace, trace_cores=trace_cores,
    )


def kernel(x, wq, bq, wk, bk, wv, bv, wo):
    in_maps = _prep_inputs(x, wq, bq, wk, bk, wv, bv, wo)
    res = _run(in_maps)
    acc = np.zeros((CIN, NPIX), np.float64)
    for r in res.results:
        acc += r["outp"].astype(np.float64)
    out = acc.astype(np.float32).reshape(CIN, H, W).transpose(1, 2, 0)
    return out[None]

